# revision 7
# baseline (speedup 1.0000x reference)
"""DNDT (deep neural decision tree) forward kernel for 8 Trainium2 NeuronCores.

Math (per batch row b of 16384):
  h[f,j]   = (x[b,f] * W[j] + bias[f,j]) / t,  W = [1..4], bias = cumsum([0,-sorted_cuts])
  bins     = softmax_j(h)                       # [6, 4]
  leaf     = kron(bins[0], ..., bins[5])        # [4096]
  out[b]   = leaf @ leaf_score                  # [10]

Device algorithm (pure data parallel, 2048 rows/core, batch-major layout
[128 partitions x 16 rows-per-partition x ...]):
  * softmax shift uses the analytic bound g(x) = (x + 3*relu(x))/t instead of a
    max-reduce (softmax is shift invariant; exp args stay <= 0), folded as
    h' = x*(W[j]-1)/t + bias[f,j]/t - 3/t*relu(x).
  * unnormalized bins E = exp(h'); leaf never materialized:
    A = bins0*bins1*bins2*bins3 kron (256, bf16), p45 = bins4*bins5 kron (16).
  * normalizer via an extra all-ones 16-col block appended to the score
    matrix: S2[u, 10*16+v] = 1, so the same mul+reduce that contracts the
    class blocks also produces sum_u A[u] * sum_v p45[v] = prod_f Z_f.
    out = O[:, :10] * recip(O[:, 10]); no separate Z/zp/zr chain.
  * matmul path in bf16 (A, at2, S2): full-rate PE with half-size LDWEIGHTS;
    rel err ~2e-3 (bf16 has the range for the e-35 normalizer tails, fp16
    does not).
  * A is transposed 128x128 at a time through the PE; 4 row-slots per group
    to cut instruction / cross-engine event count (event teardown is ~25%
    of measured exec time).
  * D = C * p45 runs on the otherwise idle GpSimd (Pool) engine, O-reduce on
    DVE: the DVE front (H/E/kron builds) is the critical path.
  * junk matmuls on x warm the PE HAM clock gate while the DVE front runs.
  * input DMAs issued from different engine sequencers (SP/ACT/DVE) so the
    ~600ns DIRECT2D issues overlap; x goes first (it gates compute start).
"""

import numpy as np

import concourse.bass as bass
import concourse.tile as tile
from concourse import bacc, mybir
from concourse.bass_utils import run_bass_kernel_spmd

N_CORES = 8
B = 16384
BC = B // N_CORES          # rows per core = 2048
P = 128                    # partitions
M = BC // P                # rows per partition = 16
NCHUNK = 2                 # pipeline chunks
CHM = M // NCHUNK          # rows per partition per chunk = 8
QS = 4                     # row-slots per transpose/matmul group
F32 = mybir.dt.float32
F32R = mybir.dt.float32r
BF16 = mybir.dt.bfloat16
N_WARM = 10                # junk matmuls to warm the PE clock gate
NBLK = 11                  # 10 class blocks + 1 all-ones normalizer block
WID = NBLK * 16            # 176
USE_POOL_D = True          # D = C * p45 on GpSimd instead of DVE


def _build_nc(neg3invt):
    nc = bacc.Bacc("TRN2", target_bir_lowering=False, debug=False,
                   num_devices=N_CORES)
    xd = nc.dram_tensor("x", [P, M * 6], F32, kind="ExternalInput")
    cstd = nc.dram_tensor("cst", [P, 2 * 24], F32, kind="ExternalInput")
    s2d = nc.dram_tensor("s2", [256, WID], BF16, kind="ExternalInput")
    idd = nc.dram_tensor("ident", [P, P], BF16, kind="ExternalInput")
    od = nc.dram_tensor("o", [P, M * 10], F32, kind="ExternalOutput")

    with tile.TileContext(nc) as tc:
        with tc.tile_pool(name="consts", bufs=1) as consts, \
             tc.tile_pool(name="work", bufs=2) as work, \
             tc.tile_pool(name="atp", bufs=2) as atp, \
             tc.tile_pool(name="ps_t", bufs=2, space="PSUM") as ps_t, \
             tc.tile_pool(name="ps_c", bufs=2, space="PSUM") as ps_c:
            x_st = consts.tile([P, M * 6], F32)
            nc.sync.dma_start(out=x_st[:], in_=xd[:])
            xv3 = x_st[:].rearrange("p (i f) -> p i f", i=M)

            # HAM warm-up: junk matmuls on the otherwise idle PE while the
            # front (DMAs, DVE H/E/kron) runs.  fp32 (slow path) on purpose:
            # more PE-busy cycles per instruction.
            def warm_mm(n):
                for _ in range(n):
                    wps = ps_t.tile([P, 8, P], F32, tag="tp")
                    nc.tensor.matmul(wps[0:M * 6, 0, 0:M * 6],
                                     lhsT=x_st[:], rhs=x_st[:, 0:M * 6],
                                     start=True, stop=True)
            warm_mm(N_WARM)

            cst_st = consts.tile([P, 2, 6, 4], F32)
            nc.sync.dma_start(out=cst_st[:].rearrange("p k f j -> p (k f j)"),
                              in_=cstd[:])
            s2_sb = consts.tile([P, 2, WID], BF16)
            nc.scalar.dma_start(out=s2_sb[:],
                                in_=s2d[:].rearrange("(k p) n -> p k n", p=P))
            ident = consts.tile([P, P], BF16)
            nc.scalar.dma_start(out=ident[:], in_=idd[:])

            for c in range(NCHUNK):
                xv = xv3[:, c * CHM:(c + 1) * CHM, :]
                # r2 = -3/t * relu(x)   (fused max+mul)
                r2 = work.tile([P, CHM, 6, 1], F32, tag="r2")
                nc.vector.tensor_scalar(out=r2[:, :, :, 0], in0=xv,
                                        scalar1=0.0, scalar2=neg3invt,
                                        op0=mybir.AluOpType.max, op1=mybir.AluOpType.mult)
                H = work.tile([P, CHM, 6, 4], F32, tag="H")
                nc.vector.tensor_mul(H[:], xv[:, :, :, None].broadcast_to((P, CHM, 6, 4)),
                                     cst_st[:, 0:1, :, :].broadcast_to((P, CHM, 6, 4)))
                nc.vector.tensor_add(H[:], H[:], cst_st[:, 1:2, :, :].broadcast_to((P, CHM, 6, 4)))
                nc.vector.tensor_add(H[:], H[:], r2[:].broadcast_to((P, CHM, 6, 4)))
                E = work.tile([P, CHM, 6, 4], F32, tag="E")
                nc.scalar.activation(E[:].rearrange("p i f j -> p (i f j)"),
                                     H[:].rearrange("p i f j -> p (i f j)"),
                                     mybir.ActivationFunctionType.Exp)

                p01 = work.tile([P, CHM, 16], F32, tag="p01")
                p23 = work.tile([P, CHM, 16], BF16, tag="p23")
                p45 = work.tile([P, CHM, 16], F32, tag="p45")
                for (pt, fa, fb) in ((p01, 0, 1), (p23, 2, 3), (p45, 4, 5)):
                    nc.vector.tensor_mul(
                        pt[:].rearrange("p i (a b) -> p i a b", a=4),
                        E[:, :, fa, :, None].broadcast_to((P, CHM, 4, 4)),
                        E[:, :, fb, None, :].broadcast_to((P, CHM, 4, 4)))
                # dense-replicate p01 over the p23 axis (cast copy: eligible for
                # the DVE 2x SBUF mode), then a dense bf16 kron-mul (2x_1p):
                # two 2x ops beat one broadcast mul at 1x.
                p01r = work.tile([P, CHM, 16, 16], BF16, tag="p01r")
                A = work.tile([P, CHM, 256], BF16, tag="A")
                for hh in range(2):
                    sl = slice(hh * (CHM // 2), (hh + 1) * (CHM // 2))
                    nc.vector.tensor_copy(
                        p01r[:, sl],
                        p01[:, sl, :, None].broadcast_to((P, CHM // 2, 16, 16)))
                    nc.vector.tensor_mul(
                        A[:, sl, :].rearrange("p i (a b) -> p i a b", a=16),
                        p01r[:, sl],
                        p23[:, sl, None, :].broadcast_to((P, CHM // 2, 16, 16)))

                Oc = work.tile([P, CHM, NBLK], F32, tag="O")
                for g in range(CHM // QS):
                    base = g * QS
                    tp = ps_t.tile([P, 2 * QS, P], BF16, tag="tp")
                    for ii in range(QS):
                        for k in range(2):
                            nc.tensor.transpose(tp[:, ii * 2 + k, :],
                                                A[:, base + ii, k * P:(k + 1) * P],
                                                ident[:])
                    at2 = atp.tile([P, 2 * QS, P], BF16, tag="at")
                    nc.scalar.copy(out=at2[:], in_=tp[:])
                    # pad each slot to 256 f32 so a slot's 176 cols never
                    # cross a 2KB PSUM bank boundary (matmul out must stay in-bank)
                    cpp = ps_c.tile([P, QS, 256], F32, tag="cp")
                    for ii in range(QS):
                        nc.tensor.matmul(cpp[:, ii, 0:WID], lhsT=at2[:, ii * 2, :],
                                         rhs=s2_sb[:, 0, :], start=True, stop=False)
                        nc.tensor.matmul(cpp[:, ii, 0:WID], lhsT=at2[:, ii * 2 + 1, :],
                                         rhs=s2_sb[:, 1, :], start=False, stop=True)
                    sl = slice(base, base + QS)
                    D = work.tile([P, QS, NBLK, 16], F32, tag="D")
                    last = (c == NCHUNK - 1) and (g == CHM // QS - 1)
                    if USE_POOL_D and not last:
                        # GpSimd can't read PSUM: stage cpp into SBUF via ACT,
                        # then the otherwise idle Pool engine does the D-mul.
                        cppsb = work.tile([P, QS, WID], F32, tag="cs")
                        nc.scalar.copy(out=cppsb[:], in_=cpp[:, :, 0:WID])
                        nc.gpsimd.tensor_mul(
                            D[:],
                            cppsb[:].rearrange("p i (c v) -> p i c v", c=NBLK),
                            p45[:, sl, None, :].broadcast_to((P, QS, NBLK, 16)))
                    else:
                        # tail group: DVE reads PSUM directly - shortest chain
                        nc.vector.tensor_mul(
                            D[:],
                            cpp[:, :, 0:WID].rearrange("p i (c v) -> p i c v", c=NBLK),
                            p45[:, sl, None, :].broadcast_to((P, QS, NBLK, 16)))
                    nc.vector.tensor_reduce(Oc[:, base:base + QS], D[:],
                                            axis=mybir.AxisListType.X,
                                            op=mybir.AluOpType.add)
                zrc = work.tile([P, CHM, 1], F32, tag="zr")
                nc.vector.reciprocal(zrc[:, :, 0], Oc[:, :, 10])
                Of = work.tile([P, CHM, 10], F32, tag="Of")
                nc.vector.tensor_mul(Of[:], Oc[:, :, 0:10],
                                     zrc[:].broadcast_to((P, CHM, 10)))
                row0 = c * CHM
                nc.sync.dma_start(
                    out=od[:].rearrange("p (i c) -> p i c", i=M)[:, row0:row0 + CHM, :],
                    in_=Of[:])
    nc.compile()
    return nc


def prep_inputs(x, cuts, leaf_score, temperature):
    """Host-side parameter prep (tiny). Returns (in_maps, invt)."""
    import ml_dtypes
    x = np.ascontiguousarray(np.asarray(x, dtype=np.float32))
    cuts = np.asarray(cuts, dtype=np.float32)
    leaf_score = np.asarray(leaf_score, dtype=np.float32)
    invt = 1.0 / float(np.asarray(temperature).reshape(-1)[0])

    sc = np.sort(cuts, axis=1)
    bias = np.cumsum(np.concatenate([np.zeros((6, 1), np.float32), -sc], axis=1,
                                    dtype=np.float32), axis=1)          # [6,4]
    W = np.arange(1.0, 5.0, dtype=np.float32)
    w2 = np.tile(((W - 1.0) * invt)[None, :], (6, 1))                    # [6,4]
    bt = bias * invt                                                     # [6,4]
    cst = np.ascontiguousarray(np.broadcast_to(
        np.stack([w2, bt]).reshape(1, 48), (P, 48)).astype(np.float32))
    s2 = np.zeros((256, WID), np.float32)
    s2[:, :160] = leaf_score.reshape(256, 16, 10).transpose(0, 2, 1).reshape(256, 160)
    s2[:, 160:] = 1.0
    s2 = np.ascontiguousarray(s2.astype(ml_dtypes.bfloat16))
    ident = np.eye(P, dtype=np.float32).astype(ml_dtypes.bfloat16)

    xs = x.reshape(N_CORES, P, M * 6)
    in_maps = [{"x": xs[i], "cst": cst, "s2": s2, "ident": ident}
               for i in range(N_CORES)]
    return in_maps, invt


_CACHE = {}


def kernel(x, cuts, leaf_score, temperature):
    in_maps, invt = prep_inputs(x, cuts, leaf_score, temperature)
    key = ("nc", float(invt))
    if key not in _CACHE:
        _CACHE[key] = _build_nc(-3.0 * invt)
        _CACHE["nc"] = _CACHE[key]
    nc = _CACHE[key]
    res = run_bass_kernel_spmd(nc, in_maps, list(range(N_CORES))).results
    out = np.concatenate([r["o"].reshape(BC, 10) for r in res], axis=0)
    return out.astype(np.float32)


# revision 8
# speedup vs baseline: 1.0235x; 1.0235x over previous
"""DNDT (deep neural decision tree) forward kernel for 8 Trainium2 NeuronCores.

Math (per batch row b of 16384):
  h[f,j]   = (x[b,f] * W[j] + bias[f,j]) / t,  W = [1..4], bias = cumsum([0,-sorted_cuts])
  bins     = softmax_j(h)                       # [6, 4]
  leaf     = kron(bins[0], ..., bins[5])        # [4096]
  out[b]   = leaf @ leaf_score                  # [10]

Device algorithm (pure data parallel, 2048 rows/core, batch-major layout
[128 partitions x 16 rows-per-partition x ...]):
  * softmax shift uses the analytic bound g(x) = (x + 3*relu(x))/t instead of a
    max-reduce (softmax is shift invariant; exp args stay <= 0), folded as
    h' = x*(W[j]-1)/t + bias[f,j]/t - 3/t*relu(x).
  * unnormalized bins E = exp(h'); leaf never materialized:
    A = bins0*bins1*bins2*bins3 kron (256, bf16), p45 = bins4*bins5 kron (16).
  * normalizer via an extra all-ones 16-col block appended to the score
    matrix: S2[u, 10*16+v] = 1, so the same mul+reduce that contracts the
    class blocks also produces sum_u A[u] * sum_v p45[v] = prod_f Z_f.
    out = O[:, :10] * recip(O[:, 10]); no separate Z/zp/zr chain.
  * matmul path in bf16 (A, at2, S2): full-rate PE with half-size LDWEIGHTS;
    rel err ~2e-3 (bf16 has the range for the e-35 normalizer tails, fp16
    does not). Each PSUM matmul slot padded to 256 f32 so its 176 cols
    never cross a 2KB PSUM bank (bank-crossing corrupts accumulation).
  * back half per 2-slot half-group: ACT stages cpp->SBUF, the otherwise
    idle GpSimd (Pool) does D = C*p45, DVE reduces O; the final half runs
    D on DVE straight from PSUM (shortest tail chain).
  * junk matmuls on x warm the PE HAM clock gate while the DVE front runs.
  * two input DMAs total (x+cst fp32 on SP, s2+ident bf16 on ACT): each
    DIRECT2D issue costs ~650ns on a sequencer and a late cst was gating
    the H chain.
"""

import numpy as np

import concourse.bass as bass
import concourse.tile as tile
from concourse import bacc, mybir
from concourse.bass_utils import run_bass_kernel_spmd

N_CORES = 8
B = 16384
BC = B // N_CORES          # rows per core = 2048
P = 128                    # partitions
M = BC // P                # rows per partition = 16
NCHUNK = 2                 # pipeline chunks
CHM = M // NCHUNK          # rows per partition per chunk = 8
QS = 4                     # row-slots per transpose/matmul group
F32 = mybir.dt.float32
BF16 = mybir.dt.bfloat16
N_WARM = 10                # junk matmuls to warm the PE clock gate
NBLK = 11                  # 10 class blocks + 1 all-ones normalizer block
WID = NBLK * 16            # 176


def _build_nc(neg3invt):
    nc = bacc.Bacc("TRN2", target_bir_lowering=False, debug=False,
                   num_devices=N_CORES)
    xd = nc.dram_tensor("xc", [P, M * 6 + 48], F32, kind="ExternalInput")
    sid = nc.dram_tensor("si", [P, 2 * WID + P], BF16, kind="ExternalInput")
    od = nc.dram_tensor("o", [P, M * 10], F32, kind="ExternalOutput")

    with tile.TileContext(nc) as tc:
        with tc.tile_pool(name="consts", bufs=1) as consts, \
             tc.tile_pool(name="work", bufs=2) as work, \
             tc.tile_pool(name="atp", bufs=2) as atp, \
             tc.tile_pool(name="ps_t", bufs=2, space="PSUM") as ps_t, \
             tc.tile_pool(name="ps_c", bufs=2, space="PSUM") as ps_c:
            xc_st = consts.tile([P, M * 6 + 48], F32)
            nc.sync.dma_start(out=xc_st[:], in_=xd[:])
            xv3 = xc_st[:, 0:M * 6].rearrange("p (i f) -> p i f", i=M)
            cst_st = xc_st[:, M * 6:].rearrange("p (k f j) -> p k f j", k=2, f=6)

            # HAM warm-up: junk matmuls on the otherwise idle PE while the
            # front (DMAs, DVE H/E/kron) runs.  fp32 (slow path) on purpose:
            # more PE-busy cycles per instruction.
            def warm_mm(n):
                for _ in range(n):
                    wps = ps_t.tile([P, 8, P], F32, tag="tp")
                    nc.tensor.matmul(wps[0:M * 6, 0, 0:M * 6],
                                     lhsT=xc_st[:, 0:M * 6], rhs=xc_st[:, 0:M * 6],
                                     start=True, stop=True)
            warm_mm(N_WARM)

            si_st = consts.tile([P, 2 * WID + P], BF16)
            nc.scalar.dma_start(out=si_st[:], in_=sid[:])
            s2_sb = si_st[:, 0:2 * WID].rearrange("p (k n) -> p k n", k=2)
            ident = si_st[:, 2 * WID:]

            for c in range(NCHUNK):
                xv = xv3[:, c * CHM:(c + 1) * CHM, :]
                # r2 = -3/t * relu(x)   (fused max+mul)
                r2 = work.tile([P, CHM, 6, 1], F32, tag="r2")
                nc.vector.tensor_scalar(out=r2[:, :, :, 0], in0=xv,
                                        scalar1=0.0, scalar2=neg3invt,
                                        op0=mybir.AluOpType.max, op1=mybir.AluOpType.mult)
                H = work.tile([P, CHM, 6, 4], F32, tag="H")
                nc.vector.tensor_mul(H[:], xv[:, :, :, None].broadcast_to((P, CHM, 6, 4)),
                                     cst_st[:, 0:1, :, :].broadcast_to((P, CHM, 6, 4)))
                nc.vector.tensor_add(H[:], H[:], cst_st[:, 1:2, :, :].broadcast_to((P, CHM, 6, 4)))
                nc.vector.tensor_add(H[:], H[:], r2[:].broadcast_to((P, CHM, 6, 4)))
                E = work.tile([P, CHM, 6, 4], F32, tag="E")
                nc.scalar.activation(E[:].rearrange("p i f j -> p (i f j)"),
                                     H[:].rearrange("p i f j -> p (i f j)"),
                                     mybir.ActivationFunctionType.Exp)

                p01 = work.tile([P, CHM, 16], F32, tag="p01")
                p23 = work.tile([P, CHM, 16], F32, tag="p23")
                p45 = work.tile([P, CHM, 16], F32, tag="p45")
                for (pt, fa, fb) in ((p01, 0, 1), (p23, 2, 3), (p45, 4, 5)):
                    nc.vector.tensor_mul(
                        pt[:].rearrange("p i (a b) -> p i a b", a=4),
                        E[:, :, fa, :, None].broadcast_to((P, CHM, 4, 4)),
                        E[:, :, fb, None, :].broadcast_to((P, CHM, 4, 4)))
                A = work.tile([P, CHM, 256], BF16, tag="A")
                qq = CHM // 4
                for hh in range(4):
                    sl = slice(hh * qq, (hh + 1) * qq)
                    nc.vector.tensor_mul(
                        A[:, sl, :].rearrange("p i (a b) -> p i a b", a=16),
                        p01[:, sl, :, None].broadcast_to((P, qq, 16, 16)),
                        p23[:, sl, None, :].broadcast_to((P, qq, 16, 16)))

                for g in range(CHM // QS):
                    base = g * QS
                    tp = ps_t.tile([P, 2 * QS, P], BF16, tag="tp")
                    for ii in range(QS):
                        for k in range(2):
                            nc.tensor.transpose(tp[:, ii * 2 + k, :],
                                                A[:, base + ii, k * P:(k + 1) * P],
                                                ident[:])
                    at2 = atp.tile([P, 2 * QS, P], BF16, tag="at")
                    # pad each slot to 256 f32 so a slot's 176 cols never
                    # cross a 2KB PSUM bank (matmul out must stay in-bank)
                    cpp = ps_c.tile([P, QS, 256], F32, tag="cp")
                    Og = work.tile([P, QS, NBLK], F32, tag="O")
                    for h in range(2):          # half-groups of 2 slots
                        hs = slice(4 * h, 4 * h + 4)
                        nc.scalar.copy(out=at2[:, hs, :], in_=tp[:, hs, :])
                        for ii in (2 * h, 2 * h + 1):
                            nc.tensor.matmul(cpp[:, ii, 0:WID], lhsT=at2[:, ii * 2, :],
                                             rhs=s2_sb[:, 0, :], start=True, stop=False)
                            nc.tensor.matmul(cpp[:, ii, 0:WID], lhsT=at2[:, ii * 2 + 1, :],
                                             rhs=s2_sb[:, 1, :], start=False, stop=True)
                        i2 = slice(2 * h, 2 * h + 2)
                        sl2 = slice(base + 2 * h, base + 2 * h + 2)
                        D = work.tile([P, 2, NBLK, 16], F32, tag="D")
                        last = (c == NCHUNK - 1) and (g == CHM // QS - 1) and h == 1
                        if not last:
                            # GpSimd can't read PSUM: stage cpp into SBUF via
                            # ACT, then the idle Pool engine does the D-mul.
                            cppsb = work.tile([P, 2, WID], F32, tag="cs")
                            nc.scalar.copy(out=cppsb[:], in_=cpp[:, i2, 0:WID])
                            nc.gpsimd.tensor_mul(
                                D[:],
                                cppsb[:].rearrange("p i (c v) -> p i c v", c=NBLK),
                                p45[:, sl2, None, :].broadcast_to((P, 2, NBLK, 16)))
                        else:
                            # tail: DVE reads PSUM directly - shortest chain
                            nc.vector.tensor_mul(
                                D[:],
                                cpp[:, i2, 0:WID].rearrange("p i (c v) -> p i c v", c=NBLK),
                                p45[:, sl2, None, :].broadcast_to((P, 2, NBLK, 16)))
                        nc.vector.tensor_reduce(Og[:, i2], D[:],
                                                axis=mybir.AxisListType.X,
                                                op=mybir.AluOpType.add)
                    zr = work.tile([P, QS, 1], F32, tag="zr")
                    nc.vector.reciprocal(zr[:, :, 0], Og[:, :, 10])
                    Of = work.tile([P, QS, 10], F32, tag="Of")
                    nc.vector.tensor_mul(Of[:], Og[:, :, 0:10],
                                         zr[:].broadcast_to((P, QS, 10)))
                    row0 = c * CHM + base
                    nc.sync.dma_start(
                        out=od[:].rearrange("p (i c) -> p i c", i=M)[:, row0:row0 + QS, :],
                        in_=Of[:])
    nc.compile()
    return nc


def prep_inputs(x, cuts, leaf_score, temperature):
    """Host-side parameter prep (tiny). Returns (in_maps, invt)."""
    import ml_dtypes
    x = np.ascontiguousarray(np.asarray(x, dtype=np.float32))
    cuts = np.asarray(cuts, dtype=np.float32)
    leaf_score = np.asarray(leaf_score, dtype=np.float32)
    invt = 1.0 / float(np.asarray(temperature).reshape(-1)[0])

    sc = np.sort(cuts, axis=1)
    bias = np.cumsum(np.concatenate([np.zeros((6, 1), np.float32), -sc], axis=1,
                                    dtype=np.float32), axis=1)          # [6,4]
    W = np.arange(1.0, 5.0, dtype=np.float32)
    w2 = np.tile(((W - 1.0) * invt)[None, :], (6, 1))                    # [6,4]
    bt = bias * invt                                                     # [6,4]
    cst = np.broadcast_to(np.stack([w2, bt]).reshape(1, 48), (P, 48))
    xs = x.reshape(N_CORES, P, M * 6)

    s2 = np.zeros((256, WID), np.float32)
    s2[:, :160] = leaf_score.reshape(256, 16, 10).transpose(0, 2, 1).reshape(256, 160)
    s2[:, 160:] = 1.0
    s2 = s2.reshape(2, P, WID)
    si = np.concatenate([s2[0], s2[1], np.eye(P, dtype=np.float32)], axis=1)
    si = np.ascontiguousarray(si.astype(ml_dtypes.bfloat16))

    in_maps = []
    for i in range(N_CORES):
        xc = np.ascontiguousarray(np.concatenate([xs[i], cst], axis=1,
                                                 dtype=np.float32))
        in_maps.append({"xc": xc, "si": si})
    return in_maps, invt


_CACHE = {}


def kernel(x, cuts, leaf_score, temperature):
    in_maps, invt = prep_inputs(x, cuts, leaf_score, temperature)
    key = ("nc", float(invt))
    if key not in _CACHE:
        _CACHE[key] = _build_nc(-3.0 * invt)
        _CACHE["nc"] = _CACHE[key]
    nc = _CACHE[key]
    res = run_bass_kernel_spmd(nc, in_maps, list(range(N_CORES))).results
    out = np.concatenate([r["o"].reshape(BC, 10) for r in res], axis=0)
    return out.astype(np.float32)
